# revision 5
# baseline (speedup 1.0000x reference)
"""Trainium2 Bass kernel for nn_BatchSplitFF (expert-choice MoE routing + FFN).

Strategy (data-parallel over batch, 1 batch per NeuronCore, 8 cores):
  - fp32 logits on PE in [es, tok] orientation (routing must match the fp32
    reference argmax; bf16 logits would flip many routing decisions).
  - routing (group max / argmax / token ids) on DVE; contribution *ranks*
    (position of each (es, group) selection among same-token selections)
    computed with one strict-upper-triangular fp32 matmul + DVE reduce.
  - dispatch: SWDGE dma_gather (transpose mode, <=512 idxs per call — HW
    limit) gathers selected token rows (bf16, 2KB) from DRAM directly into
    [d%128, d//128, slot] layout.
  - expert FFN in bf16 on PE: up-proj (f1 stationary) + relu/bias on ACT,
    down-proj (h stationary, f2 moving) -> y [token, d] in PSUM.
  - un-permute: y rows are written DENSELY (one row per (es, group)
    contribution) with plain contiguous DMAs; the routing table (token id
    per contribution, 32KB) is exported, and the host does the final
    scatter-add combine in fp32 (8.4M adds = 0.04% of the FLOPs).
    dma_scatter_add loses duplicate-row updates (verified on HW) and
    indirect scatters + HBM RMW are far more expensive than 16MB of
    dense writes.
Host side only reshapes/casts/transposes inputs and combines the output.
"""

import numpy as np
import ml_dtypes

import concourse.bass as bass
import concourse.mybir as mybir
import concourse.tile as tile
from concourse import bacc
from concourse.bass_utils import run_bass_kernel_spmd

bf16 = ml_dtypes.bfloat16
f32 = mybir.dt.float32
bfl = mybir.dt.bfloat16
i16 = mybir.dt.int16
i32 = mybir.dt.int32

DM, NE, ES, ESZ = 1024, 16, 4, 64
NES = NE * ES            # 64 (e,s) expert pairs
SEQ = 2048
G = SEQ // NE            # 128 groups per core
KT = DM // 128           # 8 contraction tiles
NCORES = 8
EG = 8                   # es-pairs per group-iteration
NCALLS = NES // EG       # 8 group-iterations
GIDX = 512               # idxs per dma_gather call (HW-validated limit)
RANKS = 16               # rank planes in the staging buffer

_CACHE = {}


def _build_program(use_gather=True, use_scatter=True):
    nc = bacc.Bacc("TRN2", target_bir_lowering=False, debug=False,
                   enable_asserts=False, num_devices=1)

    xT = nc.dram_tensor("xT", [DM, SEQ], f32, kind="ExternalInput").ap()
    # natural-layout bf16 x, pre-wrapped [(r p) d -> p r d] so SBUF-source
    # dma_gather (tpr=128) finds token i at partition i%128, rank i//128
    xbf = nc.dram_tensor("xbf", [128, SEQ // 128, DM], bfl,
                         kind="ExternalInput").ap()
    c2 = nc.dram_tensor("c2", [DM, NES], f32, kind="ExternalInput").ap()
    f1w = nc.dram_tensor("f1w", [DM, NES * ESZ], bfl, kind="ExternalInput").ap()
    f2w = nc.dram_tensor("f2w", [NES * ESZ, DM], bfl, kind="ExternalInput").ap()
    biasT = nc.dram_tensor("biasT", [ESZ, NES], f32, kind="ExternalInput").ap()
    tie128 = nc.dram_tensor("tie128", [NES, 128], f32, kind="ExternalInput").ap()
    tokid = nc.dram_tensor("tokid", [NES, SEQ], f32, kind="ExternalInput").ap()
    ident = nc.dram_tensor("ident", [NES, NES], f32, kind="ExternalInput").ap()
    stage3 = nc.dram_tensor("stage3", [NES * G, DM], bfl,
                            kind="ExternalOutput").ap()
    tids = nc.dram_tensor("tids", [NES, G], i32, kind="ExternalOutput").ap()

    with tile.TileContext(nc) as tc:
        with (
            tc.tile_pool(name="consts", bufs=1) as consts,
            tc.tile_pool(name="route", bufs=1) as route,
            tc.tile_pool(name="idxp", bufs=1) as idxp,
        ):
            # ---- constants into SBUF ----
            c_sb = consts.tile([128, KT, NES], f32)
            nc.sync.dma_start(out=c_sb[:], in_=c2.rearrange("(k p) e -> p k e", p=128))
            bias_sb = consts.tile([ESZ, NES], f32)
            nc.sync.dma_start(out=bias_sb[:], in_=biasT)
            tie_sb = consts.tile([NES, 128], f32)
            nc.sync.dma_start(out=tie_sb[:], in_=tie128)
            tokid_sb = consts.tile([NES, SEQ], f32)
            nc.sync.dma_start(out=tokid_sb[:], in_=tokid)
            ident_sb = consts.tile([NES, NES], f32)
            nc.sync.dma_start(out=ident_sb[:], in_=ident)

            logits_sb = route.tile([NES, SEQ], f32)

            # ---- phase B: fp32 logits, [es, tok] orientation ----
            with (
                tc.tile_pool(name="xtp", bufs=3) as xtp,
                tc.tile_pool(name="psB", bufs=2, space="PSUM") as psB,
            ):
                xT_r = xT.rearrange("(k p) t -> p k t", p=128)
                for tt in range(16):
                    xt_t = xtp.tile([128, KT, 128], f32)
                    nc.sync.dma_start(
                        out=xt_t[:], in_=xT_r[:, :, tt * 128:(tt + 1) * 128])
                    psum_l = psB.tile([NES, 128], f32, space="PSUM")
                    for k in range(KT):
                        nc.tensor.matmul(psum_l[:], c_sb[:, k, :], xt_t[:, k, :],
                                         start=(k == 0), stop=(k == KT - 1))
                    # add tiebreak while copying PSUM -> SBUF
                    nc.vector.tensor_tensor(
                        out=logits_sb[:, tt * 128:(tt + 1) * 128],
                        in0=psum_l[:], in1=tie_sb[:],
                        op=mybir.AluOpType.add)

                # ---- phase C: routing ----
                gmax = route.tile([NES, G], f32)
                nc.vector.tensor_reduce(
                    out=gmax[:],
                    in_=logits_sb.rearrange("e (g t) -> e g t", t=NE),
                    axis=mybir.AxisListType.X, op=mybir.AluOpType.max)
                iseq = route.tile([NES, SEQ], f32)
                nc.vector.tensor_tensor(
                    out=iseq.rearrange("e (g t) -> e g t", t=NE),
                    in0=logits_sb.rearrange("e (g t) -> e g t", t=NE),
                    in1=gmax.unsqueeze(2).to_broadcast([NES, G, NE]),
                    op=mybir.AluOpType.is_equal)
                tsel = route.tile([NES, SEQ], f32)
                nc.vector.tensor_tensor(out=tsel[:], in0=iseq[:], in1=tokid_sb[:],
                                        op=mybir.AluOpType.mult)
                tid_f = route.tile([NES, G], f32)
                nc.vector.tensor_reduce(
                    out=tid_f[:],
                    in_=tsel.rearrange("e (g t) -> e g t", t=NE),
                    axis=mybir.AxisListType.X, op=mybir.AluOpType.max)

                # export routing table for the host-side combine
                tid_i32 = route.tile([NES, G], i32)
                nc.vector.tensor_copy(out=tid_i32[:], in_=tid_f[:])
                nc.sync.dma_start(out=tids, in_=tid_i32[:])

                # gather idx tiles: transpose tid into [16, gh, es] psum layout
                psum_idx = psB.tile([16, 8, NES], f32, space="PSUM", tag="psidx")
                for gh in range(8):
                    nc.tensor.transpose(
                        out=psum_idx[:, gh, :],
                        in_=tid_f[:, gh * 16:(gh + 1) * 16],
                        identity=ident_sb[:])
                # idx_mega [128, 16 calls x 32 cols]; call h covers 4 es:
                # col j = e*8+gh, value = tid(es=4h+e, g=gh*16+p)
                idx_mega = idxp.tile([128, 16 * (GIDX // 16)], i16)
                for h in range(16):
                    nc.vector.tensor_copy(
                        out=idx_mega[0:16, h * 32:(h + 1) * 32].rearrange(
                            "p (e g) -> p e g", g=8),
                        in_=psum_idx[:, :, h * 4:(h + 1) * 4].transpose([0, 2, 1]))
                # replicate idx rows to all 128 partitions (Q7 channel reads)
                nc.sync.dma_start(out=idx_mega[16:32, :], in_=idx_mega[0:16, :])
                nc.sync.dma_start(out=idx_mega[32:64, :], in_=idx_mega[0:32, :])
                nc.sync.dma_start(out=idx_mega[64:128, :], in_=idx_mega[0:64, :])

            # x resident in SBUF as the gather source (token i at partition
            # i%128, rank i//128, 2KB per rank stripe)
            x_sb = consts.tile([128, SEQ // 128, DM], bfl)
            nc.sync.dma_start(out=x_sb[:], in_=xbf)

            # ---- phase D: per es-group FFN ----
            with (
                tc.tile_pool(name="wp", bufs=2) as wp,
                tc.tile_pool(name="sp", bufs=2) as sp,
                tc.tile_pool(name="yp", bufs=2) as yp,
                tc.tile_pool(name="hp", bufs=3) as hp,
                tc.tile_pool(name="psH", bufs=3, space="PSUM") as psH,
                tc.tile_pool(name="psY", bufs=2, space="PSUM") as psY,
            ):
                f1_r = f1w.rearrange("(k p) q -> p k q", p=128)
                f2_r = f2w.rearrange("(E f) d -> f E d", f=ESZ)
                for a in range(NCALLS):
                    f1_sb = wp.tile([128, KT, EG * ESZ], bfl, tag="f1")
                    nc.sync.dma_start(
                        out=f1_sb[:],
                        in_=f1_r[:, :, a * EG * ESZ:(a + 1) * EG * ESZ])
                    f2_sb = wp.tile([ESZ, EG, DM], bfl, tag="f2")
                    nc.sync.dma_start(
                        out=f2_sb[:], in_=f2_r[:, a * EG:(a + 1) * EG, :])

                    selT_halves = []
                    for half in range(2):
                        selTh = sp.tile([128, KT, GIDX], bfl, tag=f"selT{half}",
                                        name=f"selT_{a}_{half}")
                        if use_gather:
                            nc.gpsimd.dma_gather(
                                out_ap=selTh[:],
                                in_ap=x_sb[:],
                                idxs_ap=idx_mega[:, (2 * a + half) * 32:
                                                 (2 * a + half + 1) * 32],
                                num_idxs=GIDX, num_idxs_reg=GIDX, elem_size=DM,
                                transpose=True,
                                sbuf_tokens_per_rank=128,
                                sbuf_free_dim_per_rank=DM * 2)
                        else:
                            nc.vector.memset(selTh[:], 0)
                        selT_halves.append(selTh)

                    y_sb = yp.tile([128, EG, DM], bfl)
                    for e in range(EG):
                        es = a * EG + e
                        psum_h = psH.tile([ESZ, G], f32, space="PSUM")
                        selTh = selT_halves[e // 4]
                        eh = e % 4
                        for k in range(KT):
                            nc.tensor.matmul(
                                psum_h[:],
                                f1_sb[:, k, e * ESZ:(e + 1) * ESZ],
                                selTh[:, k, eh * G:(eh + 1) * G],
                                start=(k == 0), stop=(k == KT - 1))
                        h_sb = hp.tile([ESZ, G], bfl)
                        nc.scalar.activation(
                            out=h_sb[:], in_=psum_h[:],
                            func=mybir.ActivationFunctionType.Relu,
                            bias=bias_sb[:, es:es + 1], scale=1.0)
                        psum_y = psY.tile([128, DM], f32, space="PSUM")
                        for n in range(2):
                            nc.tensor.matmul(
                                psum_y[:, n * 512:(n + 1) * 512],
                                h_sb[:],
                                f2_sb[:, e, n * 512:(n + 1) * 512],
                                start=True, stop=True)
                        if e % 2 == 0:
                            nc.vector.tensor_copy(out=y_sb[:, e, :], in_=psum_y[:])
                        else:
                            nc.scalar.copy(out=y_sb[:, e, :], in_=psum_y[:])

                    # dense write: rows [a*1024, (a+1)*1024) = (es_local, g)
                    nc.sync.dma_start(
                        out=stage3[a * EG * G:(a + 1) * EG * G, :].rearrange(
                            "(e g) d -> g e d", g=G),
                        in_=y_sb[:])

    nc.compile()
    return nc


def _host_prep(x, controller, f1, f2, bias):
    """Returns (shared_map, per_core_maps)."""
    x = np.asarray(x, dtype=np.float32)
    c2 = np.ascontiguousarray(np.asarray(controller, np.float32).reshape(DM, NES))
    f1w = np.ascontiguousarray(np.asarray(f1, np.float32).reshape(DM, NES * ESZ)).astype(bf16)
    f2w = np.ascontiguousarray(np.asarray(f2, np.float32).reshape(NES * ESZ, DM)).astype(bf16)
    biasT = np.ascontiguousarray(np.asarray(bias, np.float32).reshape(NES, ESZ).T)
    tie = np.linspace(0.0, 1e-6, NE, dtype=np.float32)
    tie128 = np.broadcast_to(np.tile(tie, 128 // NE), (NES, 128)).copy()
    tokid = np.broadcast_to(np.arange(SEQ, dtype=np.float32), (NES, SEQ)).copy()
    ident = np.eye(NES, dtype=np.float32)
    utri = np.triu(np.ones((NES, NES), np.float32), k=1)
    shared = dict(c2=c2, f1w=f1w, f2w=f2w, biasT=biasT, tie128=tie128,
                  tokid=tokid, ident=ident, utri=utri)
    per_core = []
    for b in range(NCORES):
        xb = x[b]
        per_core.append(dict(
            xT=np.ascontiguousarray(xb.T),
            xbf=np.ascontiguousarray(
                xb.astype(bf16).reshape(SEQ // 128, 128, DM).transpose(1, 0, 2)),
        ))
    return shared, per_core


def _run(inputs, trace=False, tmpdir=None, trace_cores=None):
    if "nc" not in _CACHE:
        _CACHE["nc"] = _build_program()
    nc = _CACHE["nc"]
    shared, per_core = _host_prep(
        inputs["x"], inputs["controller"], inputs["f1"], inputs["f2"],
        inputs["bias"])
    in_maps = [dict(shared, **pc) for pc in per_core]
    res = run_bass_kernel_spmd(
        nc, in_maps, core_ids=list(range(NCORES)), trace=trace, tmpdir=tmpdir,
        trace_cores=trace_cores)
    out = np.zeros((NCORES, SEQ, DM), dtype=np.float32)
    for b in range(NCORES):
        st = np.asarray(res.results[b]["stage3"]).astype(np.float32)
        tid = np.asarray(res.results[b]["tids"]).reshape(-1)  # [es*G] token ids
        rows = tid.reshape(NES, G)
        # stage3 row (a*8 + e_l)*G + g holds y for es = a*8+e_l, group g
        np.add.at(out[b], rows.reshape(-1), st)
    return out, res


def kernel(**inputs) -> np.ndarray:
    out, _ = _run(inputs)
    return out



# revision 15
# speedup vs baseline: 1.0890x; 1.0890x over previous
"""Trainium2 Bass kernel for nn_BatchSplitFF (expert-choice MoE routing + FFN).

Strategy (data-parallel over batch, 1 batch per NeuronCore, 8 cores):
  - fp32 logits on PE in [es, tok] orientation, N=512 token chunks (routing
    must match the fp32 reference argmax; bf16 logits would flip decisions).
  - routing on DVE: group max -> one-hot iseq (== perm matrix in [es, tok]
    orientation) -> token ids exported for the host-side combine.
  - dispatch ON-CHIP via PE one-hot matmuls (no DMA gather: SWDGE descriptor
    overhead ~150us/queue dominated the old design): per 128-token tile,
    selT[dm, slot] = x_tile^T @ perm_tile. perm tiles are built on DVE from
    PE-transposed iseq slices. Two es-halves keep selT at 8MB in SBUF.
  - expert FFN in bf16 on PE with array tiling: up-proj packs es pairs into
    128x64 column tiles (2x), down-proj packs es pairs into 64x128 row tiles
    (2x). relu/bias on ScalarE.
  - un-permute: y rows are written DENSELY (one row per (es, g) contribution)
    with contiguous DMAs; the routing table (token id per contribution) is
    exported, and the host does the final scatter-add combine in fp32
    (8.4M adds = 0.04% of the FLOPs). dma_scatter_add loses duplicate-row
    updates (verified on HW) and on-chip combine requires a y transpose that
    costs as much as the dense write saves.
Host side only reshapes/casts/transposes inputs and combines the output.
"""

import numpy as np
import ml_dtypes

import concourse.bass as bass
import concourse.mybir as mybir
import concourse.tile as tile
from concourse import bacc
from concourse.bass_utils import run_bass_kernel_spmd

bf16 = ml_dtypes.bfloat16
f32 = mybir.dt.float32
bfl = mybir.dt.bfloat16
i32 = mybir.dt.int32

DM, NE, ES, ESZ = 1024, 16, 4, 64
NES = NE * ES            # 64 (e,s) expert pairs
SEQ = 2048
G = SEQ // NE            # 128 groups per core
KT = DM // 128           # 8 contraction tiles
NCORES = 8
NTT = SEQ // 128         # 16 token tiles (8 groups each)
GPT = 8                  # groups per token tile
EHALF = NES // 2         # 32 es per dispatch half
NPAIR = NES // 2         # 32 es pairs
PPH = NPAIR // 2         # 16 pairs per half

_CACHE = {}


def _build_program():
    nc = bacc.Bacc("TRN2", target_bir_lowering=False, debug=False,
                   enable_asserts=False, num_devices=1)

    xT = nc.dram_tensor("xT", [DM, SEQ], f32, kind="ExternalInput").ap()
    # natural-layout bf16 x wrapped [(r p) d -> p r d]: token t at
    # partition t%128, rank t//128 -> x_sb[:, tt, :] is a 128-token tile
    xbf = nc.dram_tensor("xbf", [128, NTT, DM], bfl, kind="ExternalInput").ap()
    c2 = nc.dram_tensor("c2", [DM, NES], f32, kind="ExternalInput").ap()
    f1w = nc.dram_tensor("f1w", [DM, NES * ESZ], bfl, kind="ExternalInput").ap()
    # f2 stacked in es pairs: [2*64 f, 32 pairs, DM]
    f2w = nc.dram_tensor("f2w", [128, NPAIR, DM], bfl, kind="ExternalInput").ap()
    # bias stacked in es pairs: [2*64 f, 32 pairs]
    bias2 = nc.dram_tensor("bias2", [128, NPAIR], f32, kind="ExternalInput").ap()
    tie512 = nc.dram_tensor("tie512", [NES, 512], f32, kind="ExternalInput").ap()
    tokid = nc.dram_tensor("tokid", [NES, SEQ], f32, kind="ExternalInput").ap()
    ident = nc.dram_tensor("ident", [NES, NES], f32, kind="ExternalInput").ap()
    # glmask[t%128, gl] = 1.0 iff (t%128)//16 == gl
    glmask = nc.dram_tensor("glmask", [128, GPT], bfl, kind="ExternalInput").ap()
    stage3 = nc.dram_tensor("stage3", [NES * G, DM], bfl,
                            kind="ExternalOutput").ap()
    tids = nc.dram_tensor("tids", [NES, G], i32, kind="ExternalOutput").ap()

    with tile.TileContext(nc) as tc:
        with (
            tc.tile_pool(name="consts", bufs=1) as consts,
            tc.tile_pool(name="route", bufs=1) as route,
        ):
            # ---- constants into SBUF ----
            c_sb = consts.tile([128, KT, NES], f32)
            nc.sync.dma_start(out=c_sb[:], in_=c2.rearrange("(k p) e -> p k e", p=128))
            bias_sb = consts.tile([128, NPAIR], f32)
            nc.sync.dma_start(out=bias_sb[:], in_=bias2)
            tie_sb = consts.tile([NES, 512], f32)
            nc.sync.dma_start(out=tie_sb[:], in_=tie512)
            tokid_sb = consts.tile([NES, SEQ], f32)
            nc.sync.dma_start(out=tokid_sb[:], in_=tokid)
            ident_sb = consts.tile([NES, NES], f32)
            nc.sync.dma_start(out=ident_sb[:], in_=ident)
            glmask_sb = consts.tile([128, GPT], bfl)
            nc.sync.dma_start(out=glmask_sb[:], in_=glmask)
            x_sb = consts.tile([128, NTT, DM], bfl)
            nc.sync.dma_start(out=x_sb[:], in_=xbf)

            logits_sb = route.tile([NES, SEQ], f32)

            # ---- phase B: fp32 logits, [es, tok], 512-token chunks ----
            with (
                tc.tile_pool(name="xtp", bufs=3) as xtp,
                tc.tile_pool(name="psB", bufs=2, space="PSUM") as psB,
            ):
                xT_r = xT.rearrange("(k p) t -> p k t", p=128)
                for tc4 in range(SEQ // 512):
                    xt_t = xtp.tile([128, KT, 512], f32)
                    nc.sync.dma_start(
                        out=xt_t[:], in_=xT_r[:, :, tc4 * 512:(tc4 + 1) * 512])
                    psum_l = psB.tile([NES, 512], f32, space="PSUM")
                    for k in range(KT):
                        nc.tensor.matmul(psum_l[:], c_sb[:, k, :], xt_t[:, k, :],
                                         start=(k == 0), stop=(k == KT - 1))
                    # add tiebreak while copying PSUM -> SBUF
                    nc.vector.tensor_tensor(
                        out=logits_sb[:, tc4 * 512:(tc4 + 1) * 512],
                        in0=psum_l[:], in1=tie_sb[:],
                        op=mybir.AluOpType.add)

            # ---- phase C: routing on DVE ----
            iseq = route.tile([NES, SEQ], f32)
            perm_all = route.tile([128, NTT, NES, GPT], bfl)
            with (
                tc.tile_pool(name="rt", bufs=1) as rt,
                tc.tile_pool(name="psC", bufs=4, space="PSUM") as psC,
            ):
                gmax = rt.tile([NES, G], f32)
                nc.vector.tensor_reduce(
                    out=gmax[:],
                    in_=logits_sb.rearrange("e (g t) -> e g t", t=NE),
                    axis=mybir.AxisListType.X, op=mybir.AluOpType.max)
                nc.vector.tensor_tensor(
                    out=iseq.rearrange("e (g t) -> e g t", t=NE),
                    in0=logits_sb.rearrange("e (g t) -> e g t", t=NE),
                    in1=gmax.unsqueeze(2).to_broadcast([NES, G, NE]),
                    op=mybir.AluOpType.is_equal)
                tsel = rt.tile([NES, SEQ], f32)
                nc.vector.tensor_tensor(out=tsel[:], in0=iseq[:], in1=tokid_sb[:],
                                        op=mybir.AluOpType.mult)
                tid_f = rt.tile([NES, G], f32)
                nc.vector.tensor_reduce(
                    out=tid_f[:],
                    in_=tsel.rearrange("e (g t) -> e g t", t=NE),
                    axis=mybir.AxisListType.X, op=mybir.AluOpType.max)
                # export routing table for the host-side combine
                tid_i32 = rt.tile([NES, G], i32)
                nc.vector.tensor_copy(out=tid_i32[:], in_=tid_f[:])
                nc.sync.dma_start(out=tids, in_=tid_i32[:])

                # perm tiles: iseq [es, tok] -> per token tile [tok, es] via
                # PE transpose, then mask by group-line to [tok, es, gl]
                for tt in range(NTT):
                    psum_t = psC.tile([128, NES], f32, space="PSUM")
                    nc.tensor.transpose(
                        out=psum_t[:],
                        in_=iseq[:, tt * 128:(tt + 1) * 128],
                        identity=ident_sb[:])
                    if tt % 2 == 0:
                        nc.scalar.copy(
                            out=perm_all[:, tt, :, :],
                            in_=psum_t.unsqueeze(2).to_broadcast([128, NES, GPT]))
                    else:
                        nc.vector.tensor_copy(
                            out=perm_all[:, tt, :, :],
                            in_=psum_t.unsqueeze(2).to_broadcast([128, NES, GPT]))
                for tt in range(NTT):
                    eng = nc.vector if tt % 2 == 0 else nc.gpsimd
                    eng.tensor_tensor(
                        out=perm_all[:, tt, :, :],
                        in0=perm_all[:, tt, :, :],
                        in1=glmask_sb.unsqueeze(1).to_broadcast([128, NES, GPT]),
                        op=mybir.AluOpType.mult)

            # ---- phase D: dispatch + FFN, one es-half at a time ----
            f1_r = f1w.rearrange("(k p) q -> p k q", p=128)
            for half in range(2):
                e0 = half * EHALF
                with (
                    tc.tile_pool(name="selp", bufs=1) as selp,
                    tc.tile_pool(name="dcp", bufs=3) as dcp,
                ):
                    # dispatch: selT[dm%128, k, es, g] for this half (8MB)
                    selT = selp.tile([128, KT, EHALF, G], bfl)
                    with tc.tile_pool(name="psD", bufs=2, space="PSUM") as psD:
                        for tt in range(NTT):
                            psum_s = psD.tile([128, KT, 256], f32, space="PSUM")
                            for k in range(KT):
                                nc.tensor.matmul(
                                    psum_s[:, k, :],
                                    x_sb[:, tt, k * 128:(k + 1) * 128],
                                    perm_all[:, tt, e0:e0 + EHALF, :],
                                    start=True, stop=True)
                            if tt % 2 == 1:
                                nc.scalar.copy(
                                    out=selT[:, :, :, tt * GPT:(tt + 1) * GPT],
                                    in_=psum_s.rearrange("p k (e g) -> p k e g",
                                                         e=EHALF))
                            else:
                                nc.vector.tensor_copy(
                                    out=selT[:, :, :, tt * GPT:(tt + 1) * GPT],
                                    in_=psum_s.rearrange("p k (e g) -> p k e g",
                                                         e=EHALF))

                    # up-proj: es pairs in 128x64 column tiles
                    h_all = dcp.tile([128, PPH, G], bfl, tag="h")
                    with (
                        tc.tile_pool(name="wp1", bufs=2) as wp1,
                        tc.tile_pool(name="psH", bufs=3, space="PSUM") as psH,
                    ):
                        for a in range(4):  # 4 es-octets per half
                            f1_sb = wp1.tile([128, KT, 8 * ESZ], bfl, tag="f1")
                            q0 = (e0 + a * 8) * ESZ
                            nc.sync.dma_start(
                                out=f1_sb[:], in_=f1_r[:, :, q0:q0 + 8 * ESZ])
                            for jj in range(4):  # pairs within octet
                                j = a * 4 + jj          # pair within half
                                jg = half * PPH + j     # global pair
                                psum_h = psH.tile([128, G], f32, space="PSUM")
                                el = jj * 2
                                for k in range(KT):
                                    nc.tensor.matmul(
                                        psum_h[0:64, :],
                                        f1_sb[:, k, el * ESZ:(el + 1) * ESZ],
                                        selT[:, k, a * 8 + el, :],
                                        start=(k == 0), stop=(k == KT - 1),
                                        tile_position=(0, 0))
                                    nc.tensor.matmul(
                                        psum_h[64:128, :],
                                        f1_sb[:, k, (el + 1) * ESZ:(el + 2) * ESZ],
                                        selT[:, k, a * 8 + el + 1, :],
                                        start=(k == 0), stop=(k == KT - 1),
                                        tile_position=(0, 64))
                                nc.scalar.activation(
                                    out=h_all[:, j, :], in_=psum_h[:],
                                    func=mybir.ActivationFunctionType.Relu,
                                    bias=bias_sb[:, jg:jg + 1], scale=1.0)

                    # down-proj: es pairs in 64x128 row tiles
                    with (
                        tc.tile_pool(name="wp2", bufs=2) as wp2,
                        tc.tile_pool(name="yp", bufs=3) as yp,
                        tc.tile_pool(name="psY", bufs=2, space="PSUM") as psY,
                    ):
                        for a in range(4):  # 4-pair f2 chunks
                            f2_sb = wp2.tile([128, 4, DM], bfl, tag="f2")
                            p0 = half * PPH + a * 4
                            nc.sync.dma_start(
                                out=f2_sb[:], in_=f2w[:, p0:p0 + 4, :])
                            for jj in range(4):
                                j = a * 4 + jj
                                jg = half * PPH + j
                                psum_y0 = psY.tile([128, DM], f32, space="PSUM",
                                                   tag="y0")
                                psum_y1 = psY.tile([128, DM], f32, space="PSUM",
                                                   tag="y1")
                                for n in range(2):
                                    nc.tensor.matmul(
                                        psum_y0[:, n * 512:(n + 1) * 512],
                                        h_all[0:64, j, :],
                                        f2_sb[0:64, jj, n * 512:(n + 1) * 512],
                                        start=True, stop=True,
                                        tile_position=(0, 0))
                                    nc.tensor.matmul(
                                        psum_y1[:, n * 512:(n + 1) * 512],
                                        h_all[64:128, j, :],
                                        f2_sb[64:128, jj, n * 512:(n + 1) * 512],
                                        start=True, stop=True,
                                        tile_position=(64, 0))
                                y_sb = yp.tile([128, 2, DM], bfl)
                                if jj % 2 == 0:
                                    nc.vector.tensor_copy(out=y_sb[:, 0, :],
                                                          in_=psum_y0[:])
                                    nc.scalar.copy(out=y_sb[:, 1, :], in_=psum_y1[:])
                                else:
                                    nc.scalar.copy(out=y_sb[:, 0, :], in_=psum_y0[:])
                                    nc.vector.tensor_copy(out=y_sb[:, 1, :],
                                                          in_=psum_y1[:])
                                # dense write: rows (es, g) for es = 2*jg, 2*jg+1
                                nc.sync.dma_start(
                                    out=stage3[2 * jg * G:(2 * jg + 2) * G, :]
                                    .rearrange("(e g) d -> g e d", g=G),
                                    in_=y_sb[:])

    nc.compile()
    return nc


def _host_prep(x, controller, f1, f2, bias):
    """Returns (shared_map, per_core_maps)."""
    x = np.asarray(x, dtype=np.float32)
    c2 = np.ascontiguousarray(np.asarray(controller, np.float32).reshape(DM, NES))
    f1w = np.ascontiguousarray(np.asarray(f1, np.float32).reshape(DM, NES * ESZ)).astype(bf16)
    # f2 stacked in es pairs: [(pair-parity f), pair, DM]
    f2p = np.asarray(f2, np.float32).reshape(NPAIR, 2, ESZ, DM)
    f2w = np.ascontiguousarray(f2p.transpose(1, 2, 0, 3).reshape(128, NPAIR, DM)).astype(bf16)
    b2 = np.asarray(bias, np.float32).reshape(NPAIR, 2, ESZ)
    bias2 = np.ascontiguousarray(b2.transpose(1, 2, 0).reshape(128, NPAIR))
    tie = np.linspace(0.0, 1e-6, NE, dtype=np.float32)
    tie512 = np.broadcast_to(np.tile(tie, 512 // NE), (NES, 512)).copy()
    tokid = np.broadcast_to(np.arange(SEQ, dtype=np.float32), (NES, SEQ)).copy()
    ident = np.eye(NES, dtype=np.float32)
    gl = (np.arange(128) // NE)[:, None] == np.arange(GPT)[None, :]
    glmask = np.ascontiguousarray(gl.astype(bf16))
    shared = dict(c2=c2, f1w=f1w, f2w=f2w, bias2=bias2, tie512=tie512,
                  tokid=tokid, ident=ident, glmask=glmask)
    per_core = []
    for b in range(NCORES):
        xb = x[b]
        per_core.append(dict(
            xT=np.ascontiguousarray(xb.T),
            xbf=np.ascontiguousarray(
                xb.astype(bf16).reshape(NTT, 128, DM).transpose(1, 0, 2)),
        ))
    return shared, per_core


def _run(inputs, trace=False, tmpdir=None, trace_cores=None):
    if "nc" not in _CACHE:
        _CACHE["nc"] = _build_program()
    nc = _CACHE["nc"]
    shared, per_core = _host_prep(
        inputs["x"], inputs["controller"], inputs["f1"], inputs["f2"],
        inputs["bias"])
    in_maps = [dict(shared, **pc) for pc in per_core]
    res = run_bass_kernel_spmd(
        nc, in_maps, core_ids=list(range(NCORES)), trace=trace, tmpdir=tmpdir,
        trace_cores=trace_cores)
    out = np.zeros((NCORES, SEQ, DM), dtype=np.float32)
    for b in range(NCORES):
        st = np.asarray(res.results[b]["stage3"]).astype(np.float32)
        tid = np.asarray(res.results[b]["tids"]).reshape(-1)  # [es*G] token ids
        rows = tid.reshape(NES, G)
        # stage3 row es*G + g holds y for (es, group g)
        np.add.at(out[b], rows.reshape(-1), st)
    return out, res


def kernel(**inputs) -> np.ndarray:
    out, _ = _run(inputs)
    return out


# revision 17
# speedup vs baseline: 1.1553x; 1.0609x over previous
"""Trainium2 Bass kernel for nn_BatchSplitFF (expert-choice MoE routing + FFN).

Strategy (data-parallel over batch, 1 batch per NeuronCore, 8 cores):
  - fp32 logits on PE in [es, tok] orientation, N=512 token chunks (routing
    must match the fp32 reference argmax; bf16 logits would flip decisions).
  - routing on DVE: group max -> one-hot iseq (== perm matrix in [es, tok]
    orientation) -> token ids exported for the host-side combine.
  - dispatch ON-CHIP via PE one-hot matmuls (no DMA gather: SWDGE descriptor
    overhead ~150us/queue dominated the old design): per 128-token tile,
    selT[dm, slot] = x_tile^T @ perm_tile. perm tiles are built on DVE from
    PE-transposed iseq slices. Two es-halves keep selT at 8MB in SBUF.
  - expert FFN in bf16 on PE with array tiling: up-proj packs es pairs into
    128x64 column tiles (2x), down-proj packs es pairs into 64x128 row tiles
    (2x). relu/bias on ScalarE.
  - un-permute: y rows are written DENSELY (one row per (es, g) contribution)
    with contiguous DMAs; the routing table (token id per contribution) is
    exported, and the host does the final scatter-add combine in fp32
    (8.4M adds = 0.04% of the FLOPs). dma_scatter_add loses duplicate-row
    updates (verified on HW) and on-chip combine requires a y transpose that
    costs as much as the dense write saves.
Host side only reshapes/casts/transposes inputs and combines the output.
"""

import numpy as np
import ml_dtypes

import concourse.bass as bass
import concourse.mybir as mybir
import concourse.tile as tile
from concourse import bacc
from concourse.bass_utils import run_bass_kernel_spmd

bf16 = ml_dtypes.bfloat16
f32 = mybir.dt.float32
bfl = mybir.dt.bfloat16
i32 = mybir.dt.int32

DM, NE, ES, ESZ = 1024, 16, 4, 64
NES = NE * ES            # 64 (e,s) expert pairs
SEQ = 2048
G = SEQ // NE            # 128 groups per core
KT = DM // 128           # 8 contraction tiles
NCORES = 8
NTT = SEQ // 128         # 16 token tiles (8 groups each)
GPT = 8                  # groups per token tile
EHALF = NES // 2         # 32 es per dispatch half
NPAIR = NES // 2         # 32 es pairs
PPH = NPAIR // 2         # 16 pairs per half

_CACHE = {}


def _build_program():
    nc = bacc.Bacc("TRN2", target_bir_lowering=False, debug=False,
                   enable_asserts=False, num_devices=1)

    xT = nc.dram_tensor("xT", [DM, SEQ], f32, kind="ExternalInput").ap()
    # natural-layout bf16 x wrapped [(r p) d -> p r d]: token t at
    # partition t%128, rank t//128 -> x_sb[:, tt, :] is a 128-token tile
    xbf = nc.dram_tensor("xbf", [128, NTT, DM], bfl, kind="ExternalInput").ap()
    c2 = nc.dram_tensor("c2", [DM, NES], f32, kind="ExternalInput").ap()
    f1w = nc.dram_tensor("f1w", [DM, NES * ESZ], bfl, kind="ExternalInput").ap()
    # f2 stacked in es pairs: [2*64 f, 32 pairs, DM]
    f2w = nc.dram_tensor("f2w", [128, NPAIR, DM], bfl, kind="ExternalInput").ap()
    # bias stacked in es pairs: [2*64 f, 32 pairs]
    bias2 = nc.dram_tensor("bias2", [128, NPAIR], f32, kind="ExternalInput").ap()
    tie512 = nc.dram_tensor("tie512", [NES, 512], f32, kind="ExternalInput").ap()
    tokid = nc.dram_tensor("tokid", [NES, SEQ], f32, kind="ExternalInput").ap()
    ident = nc.dram_tensor("ident", [NES, NES], f32, kind="ExternalInput").ap()
    # glmask[t%128, gl] = 1.0 iff (t%128)//16 == gl
    glmask = nc.dram_tensor("glmask", [128, GPT], bfl, kind="ExternalInput").ap()
    stage3 = nc.dram_tensor("stage3", [NES * G, DM], bfl,
                            kind="ExternalOutput").ap()
    tids = nc.dram_tensor("tids", [NES, G], i32, kind="ExternalOutput").ap()

    with tile.TileContext(nc) as tc:
        with (
            tc.tile_pool(name="consts", bufs=1) as consts,
            tc.tile_pool(name="route", bufs=1) as route,
        ):
            # ---- constants into SBUF ----
            c_sb = consts.tile([128, KT, NES], f32)
            nc.sync.dma_start(out=c_sb[:], in_=c2.rearrange("(k p) e -> p k e", p=128))
            bias_sb = consts.tile([128, NPAIR], f32)
            nc.sync.dma_start(out=bias_sb[:], in_=bias2)
            tie_sb = consts.tile([NES, 512], f32)
            nc.sync.dma_start(out=tie_sb[:], in_=tie512)
            tokid_sb = consts.tile([NES, SEQ], f32)
            nc.sync.dma_start(out=tokid_sb[:], in_=tokid)
            ident_sb = consts.tile([NES, NES], f32)
            nc.sync.dma_start(out=ident_sb[:], in_=ident)
            glmask_sb = consts.tile([128, GPT], bfl)
            nc.sync.dma_start(out=glmask_sb[:], in_=glmask)
            x_sb = consts.tile([128, NTT, DM], bfl)
            nc.sync.dma_start(out=x_sb[:], in_=xbf)

            logits_sb = route.tile([NES, SEQ], f32)

            # ---- phase B: fp32 logits, [es, tok], 512-token chunks ----
            with (
                tc.tile_pool(name="xtp", bufs=3) as xtp,
                tc.tile_pool(name="psB", bufs=2, space="PSUM") as psB,
            ):
                xT_r = xT.rearrange("(k p) t -> p k t", p=128)
                for tc4 in range(SEQ // 512):
                    xt_t = xtp.tile([128, KT, 512], f32)
                    nc.sync.dma_start(
                        out=xt_t[:], in_=xT_r[:, :, tc4 * 512:(tc4 + 1) * 512])
                    psum_l = psB.tile([NES, 512], f32, space="PSUM")
                    for k in range(KT):
                        nc.tensor.matmul(psum_l[:], c_sb[:, k, :], xt_t[:, k, :],
                                         start=(k == 0), stop=(k == KT - 1))
                    # add tiebreak while copying PSUM -> SBUF
                    nc.vector.tensor_tensor(
                        out=logits_sb[:, tc4 * 512:(tc4 + 1) * 512],
                        in0=psum_l[:], in1=tie_sb[:],
                        op=mybir.AluOpType.add)

            # ---- phase C: routing on DVE ----
            iseq = route.tile([NES, SEQ], f32)
            perm_all = route.tile([128, NTT, NES, GPT], bfl)
            with (
                tc.tile_pool(name="rt", bufs=1) as rt,
                tc.tile_pool(name="psC", bufs=4, space="PSUM") as psC,
            ):
                gmax = rt.tile([NES, G], f32)
                nc.vector.tensor_reduce(
                    out=gmax[:],
                    in_=logits_sb.rearrange("e (g t) -> e g t", t=NE),
                    axis=mybir.AxisListType.X, op=mybir.AluOpType.max)
                nc.vector.tensor_tensor(
                    out=iseq.rearrange("e (g t) -> e g t", t=NE),
                    in0=logits_sb.rearrange("e (g t) -> e g t", t=NE),
                    in1=gmax.unsqueeze(2).to_broadcast([NES, G, NE]),
                    op=mybir.AluOpType.is_equal)
                tsel = rt.tile([NES, SEQ], f32)
                nc.vector.tensor_tensor(out=tsel[:], in0=iseq[:], in1=tokid_sb[:],
                                        op=mybir.AluOpType.mult)
                tid_f = rt.tile([NES, G], f32)
                nc.vector.tensor_reduce(
                    out=tid_f[:],
                    in_=tsel.rearrange("e (g t) -> e g t", t=NE),
                    axis=mybir.AxisListType.X, op=mybir.AluOpType.max)
                # export routing table for the host-side combine
                tid_i32 = rt.tile([NES, G], i32)
                nc.vector.tensor_copy(out=tid_i32[:], in_=tid_f[:])
                nc.sync.dma_start(out=tids, in_=tid_i32[:])

                # perm tiles: iseq [es, tok] -> per token tile [tok, es] via
                # PE transpose, then mask by group-line to [tok, es, gl]
                for tt in range(NTT):
                    psum_t = psC.tile([128, NES], f32, space="PSUM")
                    nc.tensor.transpose(
                        out=psum_t[:],
                        in_=iseq[:, tt * 128:(tt + 1) * 128],
                        identity=ident_sb[:])
                    if tt % 2 == 0:
                        nc.scalar.copy(
                            out=perm_all[:, tt, :, :],
                            in_=psum_t.unsqueeze(2).to_broadcast([128, NES, GPT]))
                    else:
                        nc.vector.tensor_copy(
                            out=perm_all[:, tt, :, :],
                            in_=psum_t.unsqueeze(2).to_broadcast([128, NES, GPT]))
                for tt in range(NTT):
                    eng = nc.vector if tt % 2 == 0 else nc.gpsimd
                    eng.tensor_tensor(
                        out=perm_all[:, tt, :, :],
                        in0=perm_all[:, tt, :, :],
                        in1=glmask_sb.unsqueeze(1).to_broadcast([128, NES, GPT]),
                        op=mybir.AluOpType.mult)

            # ---- phase D: dispatch + FFN, one es-half at a time ----
            # pools hoisted out of the half loop: no pool open/close barriers
            f1_r = f1w.rearrange("(k p) q -> p k q", p=128)
            with (
                tc.tile_pool(name="selp", bufs=1) as selp,
                tc.tile_pool(name="dcp", bufs=1) as dcp,
                tc.tile_pool(name="wp1", bufs=2) as wp1,
                tc.tile_pool(name="wp2", bufs=2) as wp2,
                tc.tile_pool(name="yp", bufs=3) as yp,
            ):
                for half in range(2):
                    e0 = half * EHALF
                    # dispatch: selT[dm%128, k, tt, es, gl] for this half (8MB);
                    # per (k, tt) the (es, gl) block is contiguous for fast evac
                    selT = selp.tile([128, KT, NTT, EHALF, GPT], bfl, tag="selT")
                    with tc.tile_pool(name="psD", bufs=2, space="PSUM") as psD:
                      for tt in range(NTT):
                        for kq in range(2):  # k quads: 2-bank psum chunks
                            psum_s = psD.tile([128, 4, 256], f32, space="PSUM",
                                              tag=f"s{kq}")
                            for kk in range(4):
                                k = kq * 4 + kk
                                nc.tensor.matmul(
                                    psum_s[:, kk, :],
                                    x_sb[:, tt, k * 128:(k + 1) * 128],
                                    perm_all[:, tt, e0:e0 + EHALF, :],
                                    start=True, stop=True)
                            if (2 * tt + kq) % 2 == 0:
                                nc.vector.tensor_copy(
                                    out=selT[:, kq * 4:(kq + 1) * 4, tt, :, :],
                                    in_=psum_s.rearrange("p k (e g) -> p k e g",
                                                         e=EHALF))
                            else:
                                nc.scalar.copy(
                                    out=selT[:, kq * 4:(kq + 1) * 4, tt, :, :],
                                    in_=psum_s.rearrange("p k (e g) -> p k e g",
                                                         e=EHALF))

                    # up-proj: es pairs in 128x64 column tiles
                    h_all = dcp.tile([128, PPH, G], bfl, tag="h")
                    with tc.tile_pool(name="psH", bufs=4, space="PSUM") as psH:
                      for a in range(4):  # 4 es-octets per half
                        f1_sb = wp1.tile([128, KT, 8 * ESZ], bfl, tag="f1")
                        q0 = (e0 + a * 8) * ESZ
                        nc.sync.dma_start(
                            out=f1_sb[:], in_=f1_r[:, :, q0:q0 + 8 * ESZ])
                        for jj in range(4):  # pairs within octet
                            j = a * 4 + jj          # pair within half
                            jg = half * PPH + j     # global pair
                            psum_h = psH.tile([128, G], f32, space="PSUM")
                            el = jj * 2
                            for k in range(KT):
                                nc.tensor.matmul(
                                    psum_h[0:64, :],
                                    f1_sb[:, k, el * ESZ:(el + 1) * ESZ],
                                    selT[:, k, :, a * 8 + el, :],
                                    start=(k == 0), stop=(k == KT - 1),
                                    tile_position=(0, 0))
                                nc.tensor.matmul(
                                    psum_h[64:128, :],
                                    f1_sb[:, k, (el + 1) * ESZ:(el + 2) * ESZ],
                                    selT[:, k, :, a * 8 + el + 1, :],
                                    start=(k == 0), stop=(k == KT - 1),
                                    tile_position=(0, 64))
                            nc.scalar.activation(
                                out=h_all[:, j, :], in_=psum_h[:],
                                func=mybir.ActivationFunctionType.Relu,
                                bias=bias_sb[:, jg:jg + 1], scale=1.0)

                    # down-proj: es pairs in 64x128 row tiles
                    with tc.tile_pool(name="psY", bufs=2, space="PSUM") as psY:
                      for a in range(4):  # 4-pair f2 chunks
                        f2_sb = wp2.tile([128, 4, DM], bfl, tag="f2")
                        p0 = half * PPH + a * 4
                        nc.sync.dma_start(
                            out=f2_sb[:], in_=f2w[:, p0:p0 + 4, :])
                        for jj in range(4):
                            j = a * 4 + jj
                            jg = half * PPH + j
                            psum_y0 = psY.tile([128, DM], f32, space="PSUM",
                                               tag="y0")
                            psum_y1 = psY.tile([128, DM], f32, space="PSUM",
                                               tag="y1")
                            for n in range(2):
                                nc.tensor.matmul(
                                    psum_y0[:, n * 512:(n + 1) * 512],
                                    h_all[0:64, j, :],
                                    f2_sb[0:64, jj, n * 512:(n + 1) * 512],
                                    start=True, stop=True,
                                    tile_position=(0, 0))
                                nc.tensor.matmul(
                                    psum_y1[:, n * 512:(n + 1) * 512],
                                    h_all[64:128, j, :],
                                    f2_sb[64:128, jj, n * 512:(n + 1) * 512],
                                    start=True, stop=True,
                                    tile_position=(64, 0))
                            y_sb = yp.tile([128, 2, DM], bfl)
                            # split evac so DVE gets ~2/3, ScalarE ~1/3
                            nc.vector.tensor_copy(out=y_sb[:, 0, :],
                                                  in_=psum_y0[:])
                            nc.vector.tensor_copy(out=y_sb[:, 1, 0:512],
                                                  in_=psum_y1[:, 0:512])
                            nc.scalar.copy(out=y_sb[:, 1, 512:1024],
                                           in_=psum_y1[:, 512:1024])
                            # dense write: rows (es, g) for es = 2*jg, 2*jg+1
                            nc.sync.dma_start(
                                out=stage3[2 * jg * G:(2 * jg + 2) * G, :]
                                .rearrange("(e g) d -> g e d", g=G),
                                in_=y_sb[:])

    nc.compile()
    return nc


def _host_prep(x, controller, f1, f2, bias):
    """Returns (shared_map, per_core_maps)."""
    x = np.asarray(x, dtype=np.float32)
    c2 = np.ascontiguousarray(np.asarray(controller, np.float32).reshape(DM, NES))
    f1w = np.ascontiguousarray(np.asarray(f1, np.float32).reshape(DM, NES * ESZ)).astype(bf16)
    # f2 stacked in es pairs: [(pair-parity f), pair, DM]
    f2p = np.asarray(f2, np.float32).reshape(NPAIR, 2, ESZ, DM)
    f2w = np.ascontiguousarray(f2p.transpose(1, 2, 0, 3).reshape(128, NPAIR, DM)).astype(bf16)
    b2 = np.asarray(bias, np.float32).reshape(NPAIR, 2, ESZ)
    bias2 = np.ascontiguousarray(b2.transpose(1, 2, 0).reshape(128, NPAIR))
    tie = np.linspace(0.0, 1e-6, NE, dtype=np.float32)
    tie512 = np.broadcast_to(np.tile(tie, 512 // NE), (NES, 512)).copy()
    tokid = np.broadcast_to(np.arange(SEQ, dtype=np.float32), (NES, SEQ)).copy()
    ident = np.eye(NES, dtype=np.float32)
    gl = (np.arange(128) // NE)[:, None] == np.arange(GPT)[None, :]
    glmask = np.ascontiguousarray(gl.astype(bf16))
    shared = dict(c2=c2, f1w=f1w, f2w=f2w, bias2=bias2, tie512=tie512,
                  tokid=tokid, ident=ident, glmask=glmask)
    per_core = []
    for b in range(NCORES):
        xb = x[b]
        per_core.append(dict(
            xT=np.ascontiguousarray(xb.T),
            xbf=np.ascontiguousarray(
                xb.astype(bf16).reshape(NTT, 128, DM).transpose(1, 0, 2)),
        ))
    return shared, per_core


def _run(inputs, trace=False, tmpdir=None, trace_cores=None):
    if "nc" not in _CACHE:
        _CACHE["nc"] = _build_program()
    nc = _CACHE["nc"]
    shared, per_core = _host_prep(
        inputs["x"], inputs["controller"], inputs["f1"], inputs["f2"],
        inputs["bias"])
    in_maps = [dict(shared, **pc) for pc in per_core]
    res = run_bass_kernel_spmd(
        nc, in_maps, core_ids=list(range(NCORES)), trace=trace, tmpdir=tmpdir,
        trace_cores=trace_cores)
    out = np.zeros((NCORES, SEQ, DM), dtype=np.float32)
    for b in range(NCORES):
        st = np.asarray(res.results[b]["stage3"]).astype(np.float32)
        tid = np.asarray(res.results[b]["tids"]).reshape(-1)  # [es*G] token ids
        rows = tid.reshape(NES, G)
        # stage3 row es*G + g holds y for (es, group g)
        np.add.at(out[b], rows.reshape(-1), st)
    return out, res


def kernel(**inputs) -> np.ndarray:
    out, _ = _run(inputs)
    return out
